# revision 1
# baseline (speedup 1.0000x reference)
import sys, os, types

sys.path.insert(0, "/opt/trn_rl_repo")
sys.path.insert(0, "/root/.axon_site")
import numpy as np

DIM = 2048
DH = 64
H = 16
HKV = 4
G = H // HKV
RANK = 8
S = 2048
NCORES = 8
NB = S // 128  # 16 q-blocks of 128 rows
NSPAN = 4     # 4 spans of 512 over S
SPAN = 512
ND = DIM // 128  # 16 D-tiles

_CACHE = {}


def _deint_perm():
    # even dims 0,2,..62 -> rows 0..31 ; odd dims -> rows 32..63
    p = np.zeros(DH, np.int64)
    for i in range(DH // 2):
        p[i] = 2 * i
        p[32 + i] = 2 * i + 1
    return p


def _prep(inputs):
    f16 = np.float16
    x = np.asarray(inputs["x"], np.float32)[0]          # (S, D)
    xt = np.ascontiguousarray(x.T).astype(f16)          # (D, S)
    perm = _deint_perm()

    wq = np.asarray(inputs["wq"], np.float32)[perm] * 0.125   # (64, D) permuted + scale
    wk = np.asarray(inputs["wk"], np.float32)[perm]
    wv = np.asarray(inputs["wv"], np.float32)
    wq_a = np.asarray(inputs["wq_a"], np.float32)
    wk_a = np.asarray(inputs["wk_a"], np.float32)
    wv_a = np.asarray(inputs["wv_a"], np.float32)
    wq_b = np.asarray(inputs["wq_b"], np.float32).reshape(H, DH, RANK)[:, perm, :]
    wk_b = np.asarray(inputs["wk_b"], np.float32).reshape(HKV, DH, RANK)[:, perm, :]
    wv_b = np.asarray(inputs["wv_b"], np.float32).reshape(HKV, DH, RANK)

    w1t = np.ascontiguousarray(np.concatenate([wk, wv], 0).T).astype(f16)   # (D, 128)
    w2 = np.zeros((96, DIM), np.float32)   # 32-aligned blocks: k_a@0, v_a@32, q_a@64
    w2[0:8] = wk_a; w2[32:40] = wv_a; w2[64:72] = wq_a
    w2t = np.ascontiguousarray(w2.T).astype(f16)  # (D, 96)
    wqt = np.ascontiguousarray(wq.T).astype(f16)                            # (D, 64)

    def baug(wb, scale, swap):
        # wb: (nh, 64, RANK) -> per 2-head tile lhsT [72, 128]
        nh = wb.shape[0]
        out = np.zeros((nh // 2, 128, 128), np.float32)
        for m in range(nh // 2):
            for hh in range(2):
                h = 2 * m + hh
                for d in range(DH):
                    dd = (d + 32) % DH if swap else d
                    col = 64 * hh + d
                    out[m, dd, col] = 1.0
                    out[m, 64:72, col] = wb[h, dd] * scale
        return out.astype(f16)

    kba = baug(wk_b, 2.0, False)
    kbs = baug(wk_b, 2.0, True)
    qba = baug(wq_b, 0.25, False)
    qbs = baug(wq_b, 0.25, True)
    vba = baug(wv_b, 2.0, False)
    # q identity rows must also carry the 1/8? base already scaled in wqt -> identity 1.0 OK
    # BUT qba/qbs identity rows were set to 1.0 by baug: correct.

    wo = np.asarray(inputs["wo"], np.float32)             # (D, 64)
    wo_share = np.asarray(inputs["wo_share"], np.float32)  # (D, 1024)
    wc = wo_share + np.tile(wo, (1, H))
    wct = np.ascontiguousarray(wc.T).astype(f16)           # (1024, D)

    fc = np.asarray(inputs["freq_cis"], np.float32)        # (S, 32, 2)
    cos = fc[:, :, 0].T                                    # (32, S)
    sin = fc[:, :, 1].T
    crep = np.tile(cos, (4, 1)).astype(np.float32)         # (128, S) rows r -> cos[r%32]
    sr = np.concatenate([-sin, sin], 0)                    # (64, S)
    srep = np.tile(sr, (2, 1)).astype(np.float32)          # (128, S)

    tri = (np.arange(128)[:, None] <= np.arange(128)[None, :]).astype(f16)  # keep k<=q
    tri4 = np.tile(tri, (1, 4))                            # (128, 512)

    return dict(
        xt=xt, w1t=w1t, w2t=w2t, wqt=wqt,
        kba=kba, kbs=kbs, qba=qba, qbs=qbs, vba=vba,
        wct=wct, crep=crep, srep=srep, tri4=np.ascontiguousarray(tri4),
    )


def _build_program():
    import concourse.bass as bass
    import concourse.bacc as bacc
    import concourse.mybir as mybir
    from concourse import tile

    f16 = mybir.dt.float16
    f32 = mybir.dt.float32
    AF = mybir.ActivationFunctionType
    ALU = mybir.AluOpType

    nc = bacc.Bacc("TRN2", target_bir_lowering=False)

    din = {}
    def inp(name, shape, dt=f16):
        din[name] = nc.dram_tensor(name, list(shape), dt, kind="ExternalInput")
        return din[name]

    xt = inp("xt", (DIM, S))
    w1t = inp("w1t", (DIM, 128))
    w2t = inp("w2t", (DIM, 96))
    wqt = inp("wqt", (DIM, 64))
    kba = inp("kba", (2, 128, 128))
    kbs = inp("kbs", (2, 128, 128))
    qba = inp("qba", (8, 128, 128))
    qbs = inp("qbs", (8, 128, 128))
    vba = inp("vba", (2, 128, 128))
    wct = inp("wct", (H * DH, DIM))
    crep = inp("crep", (128, S), f32)
    srep = inp("srep", (128, S), f32)
    tri4 = inp("tri4", (128, 512))
    yout = nc.dram_tensor("y", [2, 128, DIM], f32, kind="ExternalOutput")

    pid = nc.partition_id()

    with tile.TileContext(nc) as tc:
        with (
            tc.tile_pool(name="const", bufs=1) as constp,
            tc.tile_pool(name="xts", bufs=2) as xtp,
            tc.tile_pool(name="wcts", bufs=2) as wctp,
            tc.tile_pool(name="pt", bufs=2) as ptp,
            tc.tile_pool(name="ev", bufs=4) as evp,
            tc.tile_pool(name="ps", bufs=2, space="PSUM") as psp,
            tc.tile_pool(name="ps1", bufs=2, space="PSUM") as ps1p,
            tc.tile_pool(name="acc", bufs=1, space="PSUM") as accp,
        ):
            # ---- persistent SBUF (shared across the 8 bodies; only one runs) ----
            w1s = constp.tile([128, ND * 128], f16, tag="w1s", name="w1s")
            w2s = constp.tile([128, ND * 96], f16, tag="w2s", name="w2s")
            wqs = constp.tile([128, ND * 64], f16, tag="wqs", name="wqs")
            for d in range(ND):
                eng = nc.sync if d % 2 == 0 else nc.gpsimd
                eng.dma_start(out=w1s[:, d * 128:(d + 1) * 128], in_=w1t[d * 128:(d + 1) * 128, :])
                eng.dma_start(out=w2s[:, d * 96:(d + 1) * 96], in_=w2t[d * 128:(d + 1) * 128, :])
                eng.dma_start(out=wqs[:, d * 64:(d + 1) * 64], in_=wqt[d * 128:(d + 1) * 128, :])
            kbas = constp.tile([128, 2 * 128], f16, tag="kbas", name="kbas")
            kbss = constp.tile([128, 2 * 128], f16, tag="kbss", name="kbss")
            vbas = constp.tile([128, 2 * 128], f16, tag="vbas", name="vbas")
            qbas = constp.tile([128, 8 * 128], f16, tag="qbas", name="qbas")
            qbss = constp.tile([128, 8 * 128], f16, tag="qbss", name="qbss")
            for m in range(2):
                nc.sync.dma_start(out=kbas[:, m * 128:(m + 1) * 128], in_=kba[m])
                nc.sync.dma_start(out=kbss[:, m * 128:(m + 1) * 128], in_=kbs[m])
                nc.sync.dma_start(out=vbas[:, m * 128:(m + 1) * 128], in_=vba[m])
            for m in range(8):
                nc.sync.dma_start(out=qbas[:, m * 128:(m + 1) * 128], in_=qba[m])
                nc.sync.dma_start(out=qbss[:, m * 128:(m + 1) * 128], in_=qbs[m])
            creps = constp.tile([128, S], f32, tag="creps", name="creps")
            sreps = constp.tile([128, S], f32, tag="sreps", name="sreps")
            nc.sync.dma_start(out=creps[:], in_=crep[:])
            nc.sync.dma_start(out=sreps[:], in_=srep[:])
            tris = constp.tile([128, 512], f16, tag="tris", name="tris")
            nc.sync.dma_start(out=tris[:], in_=tri4[:])
            ones1 = constp.tile([1, 64], f16, tag="ones1", name="ones1")
            nc.vector.memset(ones1[:], 1.0)

            ck = constp.tile([128, S], f16, tag="ck", name="ck")      # [8 a-rows; 64 base] for k
            cv = constp.tile([128, S], f16, tag="cv", name="cv")
            cq = constp.tile([128, 256], f16, tag="cq", name="cq")    # 2 blocks x 128
            kT = constp.tile([64, HKV * S], f16, tag="kT", name="kT")  # kv head at cols [kv*S, ...)
            vsb = constp.tile([128, HKV * NB * 65], f16, tag="vsb", name="vsb")
            qT = constp.tile([64, 2 * H * 128], f16, tag="qT", name="qT")  # (block bi, head h) at (bi*16+h)*128
            onorm = constp.tile([128, 8 * 256], f16, tag="onorm", name="onorm")  # m-tile: 2 heads x (2 blocks*128)


            nc.vector.memset(ck[:], 0.0)
            nc.vector.memset(cv[:], 0.0)
            nc.vector.memset(cq[:], 0.0)

            def body(c):
                blocks = [c, 15 - c]
                # ---- stage 1: projections over full S ----
                for sp in range(NSPAN):
                    xta = xtp.tile([128, ND * SPAN], f16, tag="xta", name="xta")
                    for d in range(ND):
                        eng = nc.sync if d % 2 == 0 else nc.gpsimd
                        eng.dma_start(out=xta[:, d * SPAN:(d + 1) * SPAN],
                                      in_=xt[d * 128:(d + 1) * 128, sp * SPAN:(sp + 1) * SPAN])
                    xts = [xta[:, d * SPAN:(d + 1) * SPAN] for d in range(ND)]
                    pkv = psp.tile([128, SPAN], f32, tag="ps", name="pkv")
                    pa = ps1p.tile([96, SPAN], f32, tag="ps1", name="pa")
                    for d in range(ND):
                        nc.tensor.matmul(pkv[:], w1s[:, d * 128:(d + 1) * 128], xts[d],
                                         start=(d == 0), stop=(d == ND - 1))
                    for d in range(ND):
                        nc.tensor.matmul(pa[:], w2s[:, d * 96:(d + 1) * 96], xts[d],
                                         start=(d == 0), stop=(d == ND - 1))
                    sl = slice(sp * SPAN, (sp + 1) * SPAN)
                    nc.vector.tensor_copy(ck[64:72, sl], pa[0:8, :])
                    nc.vector.tensor_copy(ck[0:64, sl], pkv[0:64, :])
                    nc.scalar.copy(cv[64:72, sl], pa[32:40, :])
                    nc.scalar.copy(cv[0:64, sl], pkv[64:128, :])
                    for bi, b in enumerate(blocks):
                        if b // 4 != sp:
                            continue
                        off = (b % 4) * 128
                        pq = ps1p.tile([64, 128], f32, tag="ps1", name="pq")
                        for d in range(ND):
                            nc.tensor.matmul(pq[:], wqs[:, d * 64:(d + 1) * 64],
                                             xta[:, d * SPAN + off: d * SPAN + off + 128],
                                             start=(d == 0), stop=(d == ND - 1))
                        nc.vector.tensor_copy(cq[0:64, bi * 128:(bi + 1) * 128], pq[:])
                        nc.vector.tensor_copy(cq[64:72, bi * 128:(bi + 1) * 128],
                                              pa[64:72, off:off + 128])

                # ---- stage 2: B-projections + rope ----
                # k: per m-tile (2 kv-heads), per span
                for m in range(2):
                    for sp in range(NSPAN):
                        sl = slice(sp * SPAN, (sp + 1) * SPAN)
                        pk = psp.tile([128, SPAN], f32, tag="ps", name="pk")
                        pks = ps1p.tile([128, SPAN], f32, tag="ps1", name="pks")
                        nc.tensor.matmul(pk[:], kbas[:, m * 128:(m + 1) * 128], ck[:, sl],
                                         start=True, stop=True)
                        nc.tensor.matmul(pks[:], kbss[:, m * 128:(m + 1) * 128], ck[:, sl],
                                         start=True, stop=True)
                        t1 = evp.tile([128, SPAN], f32, tag="t1", name="t1")
                        t2 = evp.tile([128, SPAN], f32, tag="t2", name="t2")
                        nc.vector.tensor_mul(t1[:], pk[:], creps[:, sl])
                        nc.vector.tensor_mul(t2[:], pks[:], sreps[:, sl])
                        for hh in range(2):
                            kv = 2 * m + hh
                            ko = slice(kv * S + sp * SPAN, kv * S + (sp + 1) * SPAN)
                            nc.vector.tensor_add(kT[:, ko], t1[hh * 64:hh * 64 + 64, :],
                                                 t2[hh * 64:hh * 64 + 64, :])
                # q: per block, per m-tile (2 q-heads)
                for bi in range(2):
                    bsl = slice(bi * 128, (bi + 1) * 128)
                    for m in range(8):
                        pq1 = psp.tile([128, 128], f32, tag="ps", name="pq1")
                        pq2 = ps1p.tile([128, 128], f32, tag="ps1", name="pq2")
                        nc.tensor.matmul(pq1[:], qbas[:, m * 128:(m + 1) * 128], cq[:, bsl],
                                         start=True, stop=True)
                        nc.tensor.matmul(pq2[:], qbss[:, m * 128:(m + 1) * 128], cq[:, bsl],
                                         start=True, stop=True)
                        b = blocks[bi]
                        csl = slice(b * 128, (b + 1) * 128)
                        t1 = evp.tile([128, 128], f32, tag="t1q", name="t1q")
                        t2 = evp.tile([128, 128], f32, tag="t2q", name="t2q")
                        nc.vector.tensor_mul(t1[:], pq1[:], creps[:, csl])
                        nc.vector.tensor_mul(t2[:], pq2[:], sreps[:, csl])
                        for hh in range(2):
                            h = 2 * m + hh
                            qo = slice((bi * H + h) * 128, (bi * H + h + 1) * 128)
                            nc.vector.tensor_add(qT[:, qo], t1[hh * 64:hh * 64 + 64, :],
                                                 t2[hh * 64:hh * 64 + 64, :])
                # v natural: per kv-pair m, per key-tile
                nc.vector.memset(vsb[:, 64::65], 1.0)
                for m in range(2):
                    for t in range(NB):
                        pv = psp.tile([128, 128], f32, tag="ps", name="pv")
                        nc.tensor.matmul(pv[:], cv[:, t * 128:(t + 1) * 128],
                                         vbas[:, m * 128:(m + 1) * 128], start=True, stop=True)
                        for hh in range(2):
                            kv = 2 * m + hh
                            vo = (kv * NB + t) * 65
                            nc.vector.tensor_copy(vsb[:, vo:vo + 64], pv[:, hh * 64:hh * 64 + 64])

                # ---- stage 3: attention ----
                pacct = accp.tile([128, 1024], f32, tag="pacc", name="pacc")
                if True:
                    for kv in range(HKV):
                        ko0 = kv * S
                        for bi in range(2):
                            j = blocks[bi]
                            ptall = ptp.tile([128, NB * 512], f16, tag="ptall", name="ptall")
                            for t in range(j + 1):
                                sc = psp.tile([128, 512], f32, tag="ps", name="sc")
                                for hp in range(4):
                                    h = 4 * kv + hp
                                    qo = (bi * H + h) * 128
                                    nc.tensor.matmul(
                                        sc[:, hp * 128:(hp + 1) * 128],
                                        kT[:, ko0 + t * 128: ko0 + (t + 1) * 128],
                                        qT[:, qo:qo + 128],
                                        start=True, stop=True)
                                nc.scalar.activation(ptall[:, t * 512:(t + 1) * 512], sc[:], AF.Exp)
                                if t == j:
                                    nc.vector.tensor_mul(ptall[:, t * 512:(t + 1) * 512],
                                                         ptall[:, t * 512:(t + 1) * 512], tris[:])
                            for hp in range(4):
                                ao = (bi * 4 + hp) * 128
                                for t in range(j + 1):
                                    nc.tensor.matmul(
                                        pacct[0:65, ao:ao + 128],
                                        vsb[:, (kv * NB + t) * 65:(kv * NB + t) * 65 + 65],
                                        ptall[:, t * 512 + hp * 128: t * 512 + (hp + 1) * 128],
                                        start=(t == 0), stop=(t == j))
                            # normalize 4 heads of this (kv, block)
                            bb = slice(bi * 512, bi * 512 + 512)
                            recs = evp.tile([1, 512], f16, tag="recs", name="recs")
                            with nc.allow_low_precision(reason="recip of softmax sum, fp16 ok"):
                                nc.vector.reciprocal(recs[:], pacct[64:65, bb])
                            rbc = ps1p.tile([64, 512], f32, tag="ps1", name="rbc")
                            nc.tensor.matmul(rbc[:], ones1[:], recs[:], start=True, stop=True)
                            rbs = evp.tile([64, 512], f32, tag="rbs", name="rbs")
                            nc.scalar.copy(rbs[:], rbc[:])
                            for hp in range(4):
                                h = 4 * kv + hp
                                m = h // 2
                                rows = slice(64 * (h % 2), 64 * (h % 2) + 64)
                                oo = m * 256 + bi * 128
                                nc.vector.tensor_mul(
                                    onorm[rows, oo:oo + 128],
                                    pacct[0:64, bi * 512 + hp * 128: bi * 512 + (hp + 1) * 128],
                                    rbs[:, hp * 128:(hp + 1) * 128])

                # ---- stage 4: output projection ----
                for e in range(4):
                    wte = wctp.tile([128, 8 * 512], f16, tag="wte", name="wte")
                    for m in range(8):
                        eng = nc.sync if m % 2 == 0 else nc.gpsimd
                        eng.dma_start(out=wte[:, m * 512:(m + 1) * 512],
                                      in_=wct[m * 128:(m + 1) * 128, e * 512:(e + 1) * 512])
                    for bi in range(2):
                        py = psp.tile([128, 512], f32, tag="ps", name="py")
                        for m in range(8):
                            nc.tensor.matmul(py[:], onorm[:, m * 256 + bi * 128: m * 256 + bi * 128 + 128],
                                             wte[:, m * 512:(m + 1) * 512], start=(m == 0), stop=(m == 7))
                        ye = evp.tile([128, 512], f32, tag="ye", name="ye")
                        nc.vector.tensor_copy(ye[:], py[:])
                        nc.sync.dma_start(out=yout[bi, :, e * 512:(e + 1) * 512], in_=ye[:])

            for c in range(NCORES):
                with tc.If(pid == c):
                    body(c)

    nc.finalize()
    return nc


def kernel(**inputs):
    if "nc" not in _CACHE:
        _CACHE["nc"] = _build_program()
    nc = _CACHE["nc"]
    from concourse.bass_utils import run_bass_kernel_spmd

    pre = _prep(inputs)
    in_maps = [dict(pre) for _ in range(NCORES)]
    res = run_bass_kernel_spmd(nc, in_maps, list(range(NCORES)))
    y = np.zeros((1, S, DIM), np.float32)
    for c in range(NCORES):
        yc = res.results[c]["y"]
        y[0, c * 128:(c + 1) * 128] = yc[0]
        y[0, (15 - c) * 128:(16 - c) * 128] = yc[1]
    return y



# revision 7
# speedup vs baseline: 1.0413x; 1.0413x over previous
import sys, os

sys.path.insert(0, "/opt/trn_rl_repo")
sys.path.insert(0, "/root/.axon_site")
import numpy as np

DIM = 2048
DH = 64
H = 16
HKV = 4
RANK = 8
S = 2048
NCORES = 8
NB = S // 128       # 16 q/k blocks of 128
NSPAN = 4
SPAN = 512
ND = DIM // 128     # 16 D-tiles
W2 = 88             # ka(8) | va(8) | wq(64) | qa(8)

_CACHE = {}


def _deint_perm():
    p = np.zeros(DH, np.int64)
    for i in range(DH // 2):
        p[i] = 2 * i
        p[32 + i] = 2 * i + 1
    return p


def _prep(inputs):
    f16 = np.float16
    x = np.asarray(inputs["x"], np.float32)[0]          # (S, D)
    xt = np.ascontiguousarray(x.T).astype(f16)          # (D, S)
    perm = _deint_perm()

    wq = np.asarray(inputs["wq"], np.float32)[perm] * 0.125   # (64, D) permuted + scaled
    wk = np.asarray(inputs["wk"], np.float32)[perm]
    wv = np.asarray(inputs["wv"], np.float32)
    wq_a = np.asarray(inputs["wq_a"], np.float32)
    wk_a = np.asarray(inputs["wk_a"], np.float32)
    wv_a = np.asarray(inputs["wv_a"], np.float32)
    wq_b = np.asarray(inputs["wq_b"], np.float32).reshape(H, DH, RANK)[:, perm, :]
    wk_b = np.asarray(inputs["wk_b"], np.float32).reshape(HKV, DH, RANK)[:, perm, :]
    wv_b = np.asarray(inputs["wv_b"], np.float32).reshape(HKV, DH, RANK)

    w1t = np.ascontiguousarray(np.concatenate([wk, wk_a], 0).T).astype(f16)  # (D, 72)
    w2t = np.ascontiguousarray(np.concatenate([wv, wv_a], 0).T).astype(f16)  # (D, 72)
    w3t = np.ascontiguousarray(np.concatenate([wq, wq_a], 0).T).astype(f16)  # (D, 72)

    def baug(wb, scale, swap):
        # wb: (nh, 64, RANK) -> per 2-head tile lhsT [128, 128]
        nh = wb.shape[0]
        out = np.zeros((nh // 2, 128, 128), np.float32)
        for m in range(nh // 2):
            for hh in range(2):
                h = 2 * m + hh
                for d in range(DH):
                    dd = (d + 32) % DH if swap else d
                    col = 64 * hh + d
                    out[m, dd, col] = 1.0
                    out[m, 64:72, col] = wb[h, dd] * scale
        return out.astype(f16)

    kba = baug(wk_b, 2.0, False)
    kbs = baug(wk_b, 2.0, True)
    qba = baug(wq_b, 0.25, False)
    qbs = baug(wq_b, 0.25, True)
    vba = baug(wv_b, 2.0, False)
    vbab = np.concatenate([vba[0], vba[1]], axis=1)        # (128, 256) both m-tiles

    wo = np.asarray(inputs["wo"], np.float32)              # (D, 64)
    wo_share = np.asarray(inputs["wo_share"], np.float32)  # (D, 1024)
    wc = wo_share + np.tile(wo, (1, H))
    wct = np.ascontiguousarray(wc.T).astype(f16)           # (1024, D)

    fc = np.asarray(inputs["freq_cis"], np.float32)        # (S, 32, 2)
    cos = fc[:, :, 0].T                                    # (32, S)
    sin = fc[:, :, 1].T
    crep = np.tile(cos, (4, 1)).astype(np.float32)         # (128, S)
    sr = np.concatenate([-sin, sin], 0)                    # (64, S)
    srep = np.tile(sr, (2, 1)).astype(np.float32)          # (128, S)

    tri = (np.arange(128)[:, None] <= np.arange(128)[None, :]).astype(f16)
    tri4 = np.ascontiguousarray(np.tile(tri, (1, 4)))      # (128, 512)

    base = dict(
        xt=xt, w1t=w1t, w2t=w2t, w3t=w3t,
        kba=kba, kbs=kbs, qba=qba, qbs=qbs, vbab=vbab,
        wct=wct, crep=crep, srep=srep, tri4=tri4,
    )
    # per-core q-rope tables for own blocks (c, 15-c)
    in_maps = []
    for c in range(NCORES):
        cols = np.r_[c * 128:(c + 1) * 128, (15 - c) * 128:(16 - c) * 128]
        m = dict(base)
        m["crep_q"] = np.ascontiguousarray(crep[:, cols])
        m["srep_q"] = np.ascontiguousarray(srep[:, cols])
        in_maps.append(m)
    return in_maps


def _build_program():
    import concourse.bass as bass
    import concourse.bacc as bacc
    import concourse.mybir as mybir
    from concourse import tile

    f16 = mybir.dt.float16
    f32 = mybir.dt.float32
    AF = mybir.ActivationFunctionType

    nc = bacc.Bacc("TRN2", target_bir_lowering=False)

    def inp(name, shape, dt=f16):
        return nc.dram_tensor(name, list(shape), dt, kind="ExternalInput")

    xt = inp("xt", (DIM, S))
    w1t = inp("w1t", (DIM, 72))
    w2t = inp("w2t", (DIM, 72))
    w3t = inp("w3t", (DIM, 72))
    kba = inp("kba", (2, 128, 128))
    kbs = inp("kbs", (2, 128, 128))
    qba = inp("qba", (8, 128, 128))
    qbs = inp("qbs", (8, 128, 128))
    vbab = inp("vbab", (128, 256))
    wct = inp("wct", (H * DH, DIM))
    crep = inp("crep", (128, S), f32)
    srep = inp("srep", (128, S), f32)
    crep_q = inp("crep_q", (128, 256), f32)
    srep_q = inp("srep_q", (128, 256), f32)
    tri4 = inp("tri4", (128, 512))
    yout = nc.dram_tensor("y", [2, 128, DIM], f32, kind="ExternalOutput")

    pid = nc.partition_id()

    with tile.TileContext(nc) as tc:
        with (
            tc.tile_pool(name="const", bufs=1) as constp,
            tc.tile_pool(name="xts", bufs=2) as xtp,
            tc.tile_pool(name="pt", bufs=4) as ptp,
            tc.tile_pool(name="ev", bufs=4) as evp,
            tc.tile_pool(name="ps", bufs=2, space="PSUM") as psp,      # stage psum (pkv/pa etc)
            tc.tile_pool(name="sc", bufs=2, space="PSUM") as scp,      # attention scores
            tc.tile_pool(name="acc", bufs=2, space="PSUM") as accp,    # attention PV accum
        ):
            # ---------------- persistent SBUF ----------------
            w1s = constp.tile([128, ND, 72], f16, tag="w1s", name="w1s")
            w2s = constp.tile([128, ND, 72], f16, tag="w2s", name="w2s")
            w3s = constp.tile([128, ND, 72], f16, tag="w3s", name="w3s")
            for ws, wt, eng in ((w1s, w1t, nc.sync), (w2s, w2t, nc.gpsimd), (w3s, w3t, nc.scalar)):
                for d in range(0, ND, 8):
                    eng.dma_start(out=ws[:, d:d + 8, :],
                                  in_=wt[d * 128:(d + 8) * 128, :].rearrange("(a p) n -> p a n", p=128))
            kbas = constp.tile([128, 2 * 128], f16, tag="kbas", name="kbas")
            kbss = constp.tile([128, 2 * 128], f16, tag="kbss", name="kbss")
            vbas = constp.tile([128, 256], f16, tag="vbas", name="vbas")
            qbas = constp.tile([128, 8 * 128], f16, tag="qbas", name="qbas")
            qbss = constp.tile([128, 8 * 128], f16, tag="qbss", name="qbss")
            nc.sync.dma_start(out=kbas[:].rearrange("p (a n) -> p a n", a=2), in_=kba.rearrange("a p n -> p a n"))
            nc.sync.dma_start(out=kbss[:].rearrange("p (a n) -> p a n", a=2), in_=kbs.rearrange("a p n -> p a n"))
            nc.sync.dma_start(out=vbas[:], in_=vbab[:])
            nc.gpsimd.dma_start(out=qbas[:].rearrange("p (a n) -> p a n", a=8), in_=qba.rearrange("a p n -> p a n"))
            nc.gpsimd.dma_start(out=qbss[:].rearrange("p (a n) -> p a n", a=8), in_=qbs.rearrange("a p n -> p a n"))
            creps = constp.tile([128, S], f32, tag="creps", name="creps")
            sreps = constp.tile([128, S], f32, tag="sreps", name="sreps")
            nc.sync.dma_start(out=creps[:], in_=crep[:])
            nc.gpsimd.dma_start(out=sreps[:], in_=srep[:])
            crepq = constp.tile([128, 256], f32, tag="crepq", name="crepq")
            srepq = constp.tile([128, 256], f32, tag="srepq", name="srepq")
            nc.sync.dma_start(out=crepq[:], in_=crep_q[:])
            nc.gpsimd.dma_start(out=srepq[:], in_=srep_q[:])
            tris = constp.tile([128, 512], f16, tag="tris", name="tris")
            nc.sync.dma_start(out=tris[:], in_=tri4[:])
            ones1 = constp.tile([1, 64], f16, tag="ones1", name="ones1")
            nc.vector.memset(ones1[:], 1.0)

            ck = constp.tile([128, S], f16, tag="ck", name="ck")
            cv = constp.tile([128, S], f16, tag="cv", name="cv")
            cqf = constp.tile([128, NB, 128], f16, tag="cqf", name="cqf")   # q pre-B (full S)
            cqo = constp.tile([128, 256], f16, tag="cqo", name="cqo")       # own 2 blocks
            nc.vector.memset(ck[:], 0.0)
            nc.vector.memset(cv[:], 0.0)
            nc.vector.memset(cqf[:], 0.0)
            nc.vector.memset(cqo[:], 0.0)

            kT = constp.tile([64, HKV, NB, 128], f16, tag="kT", name="kT")
            vsb = constp.tile([128, NB, HKV, 65], f16, tag="vsb", name="vsb")
            qT = constp.tile([64, 2, H, 128], f16, tag="qT", name="qT")
            onorm = constp.tile([128, 8, 256], f16, tag="onorm", name="onorm")
            nc.vector.memset(vsb[:, :, :, 64], 1.0)

            wcts = constp.tile([128, 8, DIM], f16, tag="wcts", name="wcts")

            # ---------------- stage 1: projections over full S ----------------
            for sp in range(NSPAN):
                sl = slice(sp * SPAN, (sp + 1) * SPAN)
                xta = xtp.tile([128, ND, SPAN], f16, tag="xta", name="xta")
                engs = [nc.sync, nc.gpsimd, nc.scalar, nc.sync]
                for i in range(4):
                    # 4 d-tiles per DMA: src = per-partition gather of 4 d-rows
                    engs[i].dma_start(
                        out=xta[:, 4 * i:4 * i + 4, :],
                        in_=xt[4 * i * 128:(4 * i + 4) * 128, sl].rearrange(
                            "(a p) n -> p a n", p=128))
                p1 = psp.tile([72, SPAN], f32, tag="ps", name="p1")
                p2 = psp.tile([72, SPAN], f32, tag="ps1", name="p2")
                p3 = scp.tile([72, SPAN], f32, tag="sc", name="p3")
                for d in range(ND):
                    nc.tensor.matmul(p1[:], w1s[:, d, :], xta[:, d, :],
                                     start=(d == 0), stop=(d == ND - 1))
                for d in range(ND):
                    nc.tensor.matmul(p2[:], w2s[:, d, :], xta[:, d, :],
                                     start=(d == 0), stop=(d == ND - 1))
                for d in range(ND):
                    nc.tensor.matmul(p3[:], w3s[:, d, :], xta[:, d, :],
                                     start=(d == 0), stop=(d == ND - 1))
                nc.vector.tensor_copy(ck[0:72, sl], p1[:])
                nc.scalar.copy(cv[0:72, sl], p2[:])
                nc.vector.tensor_copy(
                    cqf[0:72, 4 * sp:4 * sp + 4, :],
                    p3[:].rearrange("p (a n) -> p a n", a=4))

            # prefetch output-projection weights (after stage-1 DMAs queued)
            for i in range(4):
                eng = [nc.sync, nc.gpsimd, nc.scalar, nc.gpsimd][i]
                eng.dma_start(
                    out=wcts[:, 2 * i:2 * i + 2, :],
                    in_=wct[2 * i * 128:(2 * i + 2) * 128, :].rearrange(
                        "(a p) n -> p a n", p=128))

            # ---------------- branch A: select own q columns ----------------
            for c in range(NCORES):
                with tc.If(pid == c):
                    nc.vector.tensor_copy(
                        cqo[0:72, :],
                        cqf[0:72, c:16 - c:max(15 - 2 * c, 1), :])

            # ---------------- stage 2: B-projections + rope ----------------
            # k: per m-tile (2 kv heads), per span
            for m in range(2):
                for sp in range(NSPAN):
                    sl = slice(sp * SPAN, (sp + 1) * SPAN)
                    pk = psp.tile([128, SPAN], f32, tag="ps", name="pk")
                    pks = psp.tile([128, SPAN], f32, tag="ps1", name="pks")
                    nc.tensor.matmul(pk[:], kbas[:, m * 128:(m + 1) * 128], ck[:, sl],
                                     start=True, stop=True)
                    nc.tensor.matmul(pks[:], kbss[:, m * 128:(m + 1) * 128], ck[:, sl],
                                     start=True, stop=True)
                    t1 = evp.tile([128, SPAN], f32, tag="t1", name="t1")
                    t2 = evp.tile([128, SPAN], f32, tag="t2", name="t2")
                    nc.vector.tensor_mul(t1[:], pk[:], creps[:, sl])
                    nc.vector.tensor_mul(t2[:], pks[:], sreps[:, sl])
                    for hh in range(2):
                        kv = 2 * m + hh
                        nc.vector.tensor_add(
                            kT[:, kv, 4 * sp:4 * sp + 4, :],
                            t1[hh * 64:hh * 64 + 64, :].rearrange("p (a n) -> p a n", a=4),
                            t2[hh * 64:hh * 64 + 64, :].rearrange("p (a n) -> p a n", a=4))
            # v: per key-tile, both m at once (N=256)
            for t in range(NB):
                pv = psp.tile([128, 4, 64], f32, tag="ps", name="pv")
                nc.tensor.matmul(pv[:], cv[:, t * 128:(t + 1) * 128], vbas[:],
                                 start=True, stop=True)
                nc.scalar.copy(vsb[:, t, :, 0:64], pv[:])
            # q: per m-tile, both blocks (N=256)
            for m in range(8):
                pq1 = psp.tile([128, 256], f32, tag="ps", name="pq1")
                pq2 = psp.tile([128, 256], f32, tag="ps1", name="pq2")
                nc.tensor.matmul(pq1[:], qbas[:, m * 128:(m + 1) * 128], cqo[:],
                                 start=True, stop=True)
                nc.tensor.matmul(pq2[:], qbss[:, m * 128:(m + 1) * 128], cqo[:],
                                 start=True, stop=True)
                t1 = evp.tile([128, 256], f32, tag="t1q", name="t1q")
                t2 = evp.tile([128, 256], f32, tag="t2q", name="t2q")
                nc.vector.tensor_mul(t1[:], pq1[:], crepq[:])
                nc.vector.tensor_mul(t2[:], pq2[:], srepq[:])
                for hh in range(2):
                    h = 2 * m + hh
                    nc.vector.tensor_add(
                        qT[:, :, h, :],
                        t1[hh * 64:hh * 64 + 64, :].rearrange("p (a n) -> p a n", a=2),
                        t2[hh * 64:hh * 64 + 64, :].rearrange("p (a n) -> p a n", a=2))

            # ---------------- stage 3: attention (branch B) ----------------
            def attention(c):
                blocks = [c, 15 - c]
                for kv in range(HKV):
                    for bi in range(2):
                        j = blocks[bi]
                        pacct = accp.tile([65, 512], f32, tag="pacc", name="pacc")
                        # software pipeline: scores run 2 ahead of PV
                        scs = {}
                        pts = {}

                        def emit_scores(t):
                            sct = scp.tile([128, 512], f32, tag="sc", name="sc")
                            nc.tensor.matmul(
                                sct[:], kT[:, kv, t, :], qT[:, bi, 4 * kv:4 * kv + 4, :],
                                start=True, stop=True)
                            scs[t] = sct

                        def emit_exp(t):
                            ptt = ptp.tile([128, 512], f16, tag="ptall", name="ptall")
                            nc.scalar.activation(ptt[:], scs.pop(t)[:], AF.Exp)
                            if t == j:
                                nc.vector.tensor_mul(ptt[:], ptt[:], tris[:])
                            pts[t] = ptt

                        def emit_pv(t):
                            nc.tensor.matmul(
                                pacct[:], vsb[:, t, kv, :], pts.pop(t)[:],
                                start=(t == 0), stop=(t == j))

                        emit_scores(0)
                        emit_exp(0)
                        if j >= 1:
                            emit_scores(1)
                            emit_exp(1)
                        for t in range(j + 1):
                            emit_pv(t)
                            if t + 2 <= j:
                                emit_scores(t + 2)
                                emit_exp(t + 2)
                        # normalization for this (kv, bi): 4 heads
                        recs = evp.tile([1, 512], f16, tag="recs", name="recs")
                        with nc.allow_low_precision(reason="softmax sum recip fp16"):
                            nc.vector.reciprocal(recs[:], pacct[64:65, :])
                        rbc = scp.tile([64, 512], f32, tag="sc", name="rbc")
                        nc.tensor.matmul(rbc[:], ones1[:], recs[:], start=True, stop=True)
                        rbs = evp.tile([64, 512], f32, tag="rbs", name="rbs")
                        nc.scalar.copy(rbs[:], rbc[:])
                        pacc4 = pacct[0:64, :].rearrange("p (a n) -> p a n", a=4)
                        rbs4 = rbs.rearrange("p (a n) -> p a n", a=4)
                        for par in range(2):  # head parity within pacct cols
                            nc.vector.tensor_mul(
                                onorm[64 * par:64 * par + 64, 2 * kv:2 * kv + 2,
                                      bi * 128:(bi + 1) * 128],
                                pacc4[:, par::2, :],
                                rbs4[:, par::2, :])

            for c in range(NCORES):
                with tc.If(pid == c):
                    attention(c)

            # ---------------- stage 4: output projection ----------------
            for e in range(4):
                for bi in range(2):
                    py = psp.tile([128, 512], f32, tag="ps", name="py")
                    for m in range(8):
                        nc.tensor.matmul(
                            py[:], onorm[:, m, bi * 128:(bi + 1) * 128],
                            wcts[:, m, e * 512:(e + 1) * 512],
                            start=(m == 0), stop=(m == 7))
                    ye = evp.tile([128, 512], f32, tag="ye", name="ye")
                    if (e + bi) % 2 == 0:
                        nc.vector.tensor_copy(ye[:], py[:])
                    else:
                        nc.scalar.copy(ye[:], py[:])
                    eng = nc.sync if bi == 0 else nc.gpsimd
                    eng.dma_start(out=yout[bi, :, e * 512:(e + 1) * 512], in_=ye[:])

    nc.finalize()
    return nc


def kernel(**inputs):
    if "nc" not in _CACHE:
        _CACHE["nc"] = _build_program()
    nc = _CACHE["nc"]
    from concourse.bass_utils import run_bass_kernel_spmd

    in_maps = _prep(inputs)
    res = run_bass_kernel_spmd(nc, in_maps, list(range(NCORES)))
    y = np.zeros((1, S, DIM), np.float32)
    for c in range(NCORES):
        yc = res.results[c]["y"]
        y[0, c * 128:(c + 1) * 128] = yc[0]
        y[0, (15 - c) * 128:(16 - c) * 128] = yc[1]
    return y


# revision 9
# speedup vs baseline: 1.3163x; 1.2640x over previous
import sys, os

sys.path.insert(0, "/opt/trn_rl_repo")
sys.path.insert(0, "/root/.axon_site")
import numpy as np

DIM = 2048
DH = 64
H = 16
HKV = 4
RANK = 8
S = 2048
NCORES = 8
NB = S // 128
NSPAN = 4
SPAN = 512
ND = DIM // 128

_CACHE = {}


def _deint_perm():
    p = np.zeros(DH, np.int64)
    for i in range(DH // 2):
        p[i] = 2 * i
        p[32 + i] = 2 * i + 1
    return p


def _tile128(a):
    n, w = a.shape
    nd = n // 128
    return np.ascontiguousarray(
        a.reshape(nd, 128, w).transpose(1, 0, 2).reshape(128, nd * w))


def _prep(inputs):
    f16 = np.float16
    x = np.asarray(inputs["x"], np.float32)[0]
    perm = _deint_perm()

    xtb = np.ascontiguousarray(
        x.reshape(S, ND, 128).transpose(2, 1, 0).reshape(128, ND * S)).astype(f16)

    wq = np.asarray(inputs["wq"], np.float32)[perm] * 0.125
    wk = np.asarray(inputs["wk"], np.float32)[perm]
    wv = np.asarray(inputs["wv"], np.float32)
    wq_a = np.asarray(inputs["wq_a"], np.float32)
    wk_a = np.asarray(inputs["wk_a"], np.float32)
    wv_a = np.asarray(inputs["wv_a"], np.float32)
    wq_b = np.asarray(inputs["wq_b"], np.float32).reshape(H, DH, RANK)[:, perm, :]
    wk_b = np.asarray(inputs["wk_b"], np.float32).reshape(HKV, DH, RANK)[:, perm, :]
    wv_b = np.asarray(inputs["wv_b"], np.float32).reshape(HKV, DH, RANK)

    w1b = _tile128(np.concatenate([wk, wk_a], 0).T.copy()).astype(f16)
    w2b = _tile128(np.concatenate([wv, wv_a], 0).T.copy()).astype(f16)
    w3b = _tile128(np.concatenate([wq, wq_a], 0).T.copy()).astype(f16)

    def baug(wb, scale, swap):
        nh = wb.shape[0]
        out = np.zeros((nh // 2, 128, 128), np.float32)
        for m in range(nh // 2):
            for hh in range(2):
                h = 2 * m + hh
                for d in range(DH):
                    dd = (d + 32) % DH if swap else d
                    col = 64 * hh + d
                    out[m, dd, col] = 1.0
                    out[m, 64:72, col] = wb[h, dd] * scale
        return np.ascontiguousarray(np.concatenate(list(out), axis=1)).astype(f16)

    kbab = baug(wk_b, 2.0, False)
    kbsb = baug(wk_b, 2.0, True)
    qbab = baug(wq_b, 0.25, False)
    qbsb = baug(wq_b, 0.25, True)
    vbab = baug(wv_b, 2.0, False)

    wo = np.asarray(inputs["wo"], np.float32)
    wo_share = np.asarray(inputs["wo_share"], np.float32)
    wc = wo_share + np.tile(wo, (1, H))
    wctb = _tile128(np.ascontiguousarray(wc.T)).astype(f16)

    fc = np.asarray(inputs["freq_cis"], np.float32)
    cos = fc[:, :, 0].T
    sin = fc[:, :, 1].T
    crep = np.tile(cos, (4, 1)).astype(np.float32)
    sr = np.concatenate([-sin, sin], 0)
    srep = np.tile(sr, (2, 1)).astype(np.float32)

    tri = (np.arange(128)[:, None] <= np.arange(128)[None, :]).astype(f16)
    tri4 = np.ascontiguousarray(np.tile(tri, (1, 4)))

    base = dict(
        xtb=xtb, w1b=w1b, w2b=w2b, w3b=w3b,
        kbab=kbab, kbsb=kbsb, qbab=qbab, qbsb=qbsb, vbab=vbab,
        wctb=wctb, crep=crep, srep=srep, tri4=tri4,
    )
    in_maps = []
    for c in range(NCORES):
        cols = np.r_[c * 128:(c + 1) * 128, (15 - c) * 128:(16 - c) * 128]
        m = dict(base)
        m["crep_q"] = np.ascontiguousarray(crep[:, cols])
        m["srep_q"] = np.ascontiguousarray(srep[:, cols])
        in_maps.append(m)
    return in_maps


def _build_program():
    import concourse.bass as bass
    import concourse.bacc as bacc
    import concourse.mybir as mybir
    from concourse import tile

    f16 = mybir.dt.float16
    f32 = mybir.dt.float32
    AF = mybir.ActivationFunctionType

    nc = bacc.Bacc("TRN2", target_bir_lowering=False)

    def inp(name, shape, dt=f16):
        return nc.dram_tensor(name, list(shape), dt, kind="ExternalInput")

    xtb = inp("xtb", (128, ND * S))
    w1b = inp("w1b", (128, ND * 72))
    w2b = inp("w2b", (128, ND * 72))
    w3b = inp("w3b", (128, ND * 72))
    kbab = inp("kbab", (128, 256))
    kbsb = inp("kbsb", (128, 256))
    qbab = inp("qbab", (128, 1024))
    qbsb = inp("qbsb", (128, 1024))
    vbab = inp("vbab", (128, 256))
    wctb = inp("wctb", (128, 8 * DIM))
    crep = inp("crep", (128, S), f32)
    srep = inp("srep", (128, S), f32)
    crep_q = inp("crep_q", (128, 256), f32)
    srep_q = inp("srep_q", (128, 256), f32)
    tri4 = inp("tri4", (128, 512))
    yout = nc.dram_tensor("y", [2, 128, DIM], f32, kind="ExternalOutput")

    pid = nc.partition_id()
    QS = [None]

    with tile.TileContext(nc) as tc:
        qs_list = [None]

        def dq():
            QS[0] = 0 if QS[0] is None else (QS[0] + 1) % 3
            return qs_list[0][QS[0]]

        with (
            tc.tile_pool(name="const", bufs=1) as constp,
            tc.tile_pool(name="xts", bufs=2) as xtp,
            tc.tile_pool(name="pt", bufs=8) as ptp,
            tc.tile_pool(name="ev", bufs=4) as evp,
            tc.tile_pool(name="ps", bufs=2, space="PSUM") as psp,
            tc.tile_pool(name="ps1", bufs=2, space="PSUM") as ps1p,
            tc.tile_pool(name="sc", bufs=2, space="PSUM") as scp,
            tc.tile_pool(name="acc", bufs=2, space="PSUM") as accp,
        ):
            qs_list[0] = [nc.sync, nc.gpsimd, nc.scalar]

            # ---- weights first (stage-1 gate), split for DMA-engine parallelism
            w1s = constp.tile([128, ND, 72], f16, tag="w1s", name="w1s")
            w2s = constp.tile([128, ND, 72], f16, tag="w2s", name="w2s")
            w3s = constp.tile([128, ND, 72], f16, tag="w3s", name="w3s")
            for ws, wb in ((w1s, w1b), (w2s, w2b), (w3s, w3b)):
                for ch in range(4):
                    dq().dma_start(out=ws[:, 4 * ch:4 * ch + 4, :],
                                   in_=wb[:, ch * 288:(ch + 1) * 288].rearrange(
                                       "p (a n) -> p a n", a=4))

            kbas = constp.tile([128, 256], f16, tag="kbas", name="kbas")
            kbss = constp.tile([128, 256], f16, tag="kbss", name="kbss")
            dq().dma_start(out=kbas[:], in_=kbab[:])
            dq().dma_start(out=kbss[:], in_=kbsb[:])
            creps = constp.tile([128, S], f32, tag="creps", name="creps")
            sreps = constp.tile([128, S], f32, tag="sreps", name="sreps")
            for hf in range(4):
                sl = slice(hf * 512, (hf + 1) * 512)
                dq().dma_start(out=creps[:, sl], in_=crep[:, sl])
                dq().dma_start(out=sreps[:, sl], in_=srep[:, sl])

            ones1 = constp.tile([1, 64], f16, tag="ones1", name="ones1")
            nc.vector.memset(ones1[:], 1.0)
            ck = constp.tile([128, S], f16, tag="ck", name="ck")
            cv = constp.tile([128, S], f16, tag="cv", name="cv")
            cqf = constp.tile([128, NB, 128], f16, tag="cqf", name="cqf")
            cqo = constp.tile([128, 256], f16, tag="cqo", name="cqo")
            nc.vector.memset(ck[:], 0.0)
            nc.vector.memset(cv[:], 0.0)
            nc.vector.memset(cqf[:], 0.0)
            nc.vector.memset(cqo[:], 0.0)

            kT = constp.tile([64, HKV, NB, 128], f16, tag="kT", name="kT")
            vsb = constp.tile([128, NB, HKV, 65], f16, tag="vsb", name="vsb")
            qT = constp.tile([64, 2, H, 128], f16, tag="qT", name="qT")
            onorm = constp.tile([128, 8, 256], f16, tag="onorm", name="onorm")
            nc.vector.memset(vsb[:, :, :, 64], 1.0)

            vbas = constp.tile([128, 256], f16, tag="vbas", name="vbas")
            qbas = constp.tile([128, 1024], f16, tag="qbas", name="qbas")
            qbss = constp.tile([128, 1024], f16, tag="qbss", name="qbss")
            crepq = constp.tile([128, 256], f32, tag="crepq", name="crepq")
            srepq = constp.tile([128, 256], f32, tag="srepq", name="srepq")
            tris = constp.tile([128, 512], f16, tag="tris", name="tris")
            wcts = constp.tile([128, 8, DIM], f16, tag="wcts", name="wcts")

            # ---- stage 1 + k-projection/rope pipelined per span ----
            for sp in range(NSPAN):
                sl = slice(sp * SPAN, (sp + 1) * SPAN)
                xta = xtp.tile([128, ND, SPAN], f16, tag="xta", name="xta")
                for d in range(ND):
                    dq().dma_start(out=xta[:, d, :],
                                   in_=xtb[:, d * S + sp * SPAN: d * S + (sp + 1) * SPAN])
                if sp == 0:
                    # queue the small late-use consts behind span-0 data
                    dq().dma_start(out=vbas[:], in_=vbab[:])
                    dq().dma_start(out=qbas[:], in_=qbab[:])
                    dq().dma_start(out=qbss[:], in_=qbsb[:])
                    dq().dma_start(out=crepq[:], in_=crep_q[:])
                    dq().dma_start(out=srepq[:], in_=srep_q[:])
                    dq().dma_start(out=tris[:], in_=tri4[:])
                p1 = psp.tile([72, SPAN], f32, tag="ps", name="p1")
                p2 = ps1p.tile([72, SPAN], f32, tag="ps1", name="p2")
                p3 = scp.tile([72, SPAN], f32, tag="sc", name="p3")
                for d in range(ND):
                    nc.tensor.matmul(p1[:], w1s[:, d, :], xta[:, d, :],
                                     start=(d == 0), stop=(d == ND - 1))
                for d in range(ND):
                    nc.tensor.matmul(p2[:], w2s[:, d, :], xta[:, d, :],
                                     start=(d == 0), stop=(d == ND - 1))
                for d in range(ND):
                    nc.tensor.matmul(p3[:], w3s[:, d, :], xta[:, d, :],
                                     start=(d == 0), stop=(d == ND - 1))
                nc.vector.tensor_copy(ck[0:72, sl], p1[:])
                nc.scalar.copy(cv[0:72, sl], p2[:])
                nc.vector.tensor_copy(
                    cqf[0:72, 4 * sp:4 * sp + 4, :],
                    p3[:].rearrange("p (a n) -> p a n", a=4))
                # k B-projection + rope for this span
                for m in range(2):
                    pk = accp.tile([128, SPAN], f32, tag="pacc", name="pk")
                    pks = accp.tile([128, SPAN], f32, tag="pacc", name="pks")
                    nc.tensor.matmul(pk[:], kbas[:, m * 128:(m + 1) * 128], ck[:, sl],
                                     start=True, stop=True)
                    nc.tensor.matmul(pks[:], kbss[:, m * 128:(m + 1) * 128], ck[:, sl],
                                     start=True, stop=True)
                    t1 = evp.tile([128, SPAN], f16, tag="t1", name="t1")
                    t2 = evp.tile([128, SPAN], f16, tag="t2", name="t2")
                    nc.vector.tensor_mul(t1[:], pk[:], creps[:, sl])
                    nc.vector.tensor_mul(t2[:], pks[:], sreps[:, sl])
                    for hh in range(2):
                        kv = 2 * m + hh
                        nc.vector.tensor_add(
                            kT[:, kv, 4 * sp:4 * sp + 4, :],
                            t1[hh * 64:hh * 64 + 64, :].rearrange("p (a n) -> p a n", a=4),
                            t2[hh * 64:hh * 64 + 64, :].rearrange("p (a n) -> p a n", a=4))

            # ---- branch A: select own q columns ----
            for c in range(NCORES):
                with tc.If(pid == c):
                    nc.vector.tensor_copy(
                        cqo[0:72, :],
                        cqf[0:72, c:16 - c:max(15 - 2 * c, 1), :])

            # ---- v + q projections ----
            for t in range(NB):
                pv = psp.tile([128, 4, 64], f32, tag="ps", name="pv")
                nc.tensor.matmul(pv[:], cv[:, t * 128:(t + 1) * 128], vbas[:],
                                 start=True, stop=True)
                nc.scalar.copy(vsb[:, t, :, 0:64], pv[:])
            for m in range(8):
                pq1 = ps1p.tile([128, 256], f32, tag="ps1", name="pq1")
                pq2 = scp.tile([128, 256], f32, tag="sc", name="pq2")
                nc.tensor.matmul(pq1[:], qbas[:, m * 128:(m + 1) * 128], cqo[:],
                                 start=True, stop=True)
                nc.tensor.matmul(pq2[:], qbss[:, m * 128:(m + 1) * 128], cqo[:],
                                 start=True, stop=True)
                t1 = evp.tile([128, 256], f16, tag="t1q", name="t1q")
                t2 = evp.tile([128, 256], f16, tag="t2q", name="t2q")
                nc.vector.tensor_mul(t1[:], pq1[:], crepq[:])
                nc.vector.tensor_mul(t2[:], pq2[:], srepq[:])
                for hh in range(2):
                    h = 2 * m + hh
                    nc.vector.tensor_add(
                        qT[:, :, h, :],
                        t1[hh * 64:hh * 64 + 64, :].rearrange("p (a n) -> p a n", a=2),
                        t2[hh * 64:hh * 64 + 64, :].rearrange("p (a n) -> p a n", a=2))

            # ---- wct prefetch (needed only in stage 4) ----
            wflat = wcts[:].rearrange("p a n -> p (a n)")
            for i in range(16):
                dq().dma_start(out=wflat[:, i * 1024:(i + 1) * 1024],
                               in_=wctb[:, i * 1024:(i + 1) * 1024])

            # ---- stage 3: attention ----
            def attention(c):
                blocks = [c, 15 - c]
                scn = [0]
                scpools = [(scp, "sc"), (psp, "ps"), (ps1p, "ps1")]

                for kv in range(HKV):
                    seq = []
                    for t in range(blocks[1] + 1):
                        if t <= blocks[0]:
                            seq.append((0, t))
                        seq.append((1, t))
                    pacct = {}
                    for bi in range(2):
                        pacct[bi] = accp.tile([65, 512], f32, tag="pacc", name="pacc")
                    pts = {}

                    def emit_se(i):
                        bi, t = seq[i]
                        pool, tag = scpools[scn[0] % 3]
                        scn[0] += 1
                        sct = pool.tile([128, 512], f32, tag=tag, name="sct")
                        nc.tensor.matmul(
                            sct[:], kT[:, kv, t, :], qT[:, bi, 4 * kv:4 * kv + 4, :],
                            start=True, stop=True)
                        ptt = ptp.tile([128, 512], f16, tag="ptall", name="ptall")
                        nc.scalar.activation(ptt[:], sct[:], AF.Exp)
                        if t == blocks[bi]:
                            nc.vector.tensor_mul(ptt[:], ptt[:], tris[:])
                        pts[i] = ptt

                    def emit_pv(i):
                        bi, t = seq[i]
                        nc.tensor.matmul(
                            pacct[bi][:], vsb[:, t, kv, :], pts.pop(i)[:],
                            start=(t == 0), stop=(t == blocks[bi]))

                    LA = 5
                    for i in range(min(LA, len(seq))):
                        emit_se(i)
                    for i in range(len(seq)):
                        emit_pv(i)
                        if i + LA < len(seq):
                            emit_se(i + LA)

                    recs = {}
                    for bi in range(2):
                        rs = evp.tile([1, 512], f32, tag="rsum", name="rsum")
                        nc.vector.tensor_copy(rs[:], pacct[bi][64:65, :])
                        r32 = evp.tile([1, 512], f32, tag="recs32", name="recs32")
                        nc.vector.reciprocal_approx_fast(out=r32[:], in_=rs[:])
                        r16 = evp.tile([1, 512], f16, tag="recs16", name="recs16")
                        with nc.allow_low_precision(reason="softmax recip bc"):
                            nc.vector.tensor_copy(r16[:], r32[:])
                        recs[bi] = r16
                    for bi in range(2):
                        pool, tag = scpools[scn[0] % 3]
                        scn[0] += 1
                        rbc = pool.tile([64, 512], f32, tag=tag, name="rbc")
                        nc.tensor.matmul(rbc[:], ones1[:], recs[bi][:], start=True, stop=True)
                        rbs = evp.tile([64, 512], f16, tag="rbs", name="rbs")
                        with nc.allow_low_precision(reason="softmax recip bc"):
                            nc.vector.tensor_copy(rbs[:], rbc[:])
                        pacc4 = pacct[bi][0:64, :].rearrange("p (a n) -> p a n", a=4)
                        rbs4 = rbs.rearrange("p (a n) -> p a n", a=4)
                        for par in range(2):
                            nc.vector.tensor_mul(
                                onorm[64 * par:64 * par + 64, 2 * kv:2 * kv + 2,
                                      bi * 128:(bi + 1) * 128],
                                pacc4[:, par::2, :],
                                rbs4[:, par::2, :])

            for c in range(NCORES):
                with tc.If(pid == c):
                    attention(c)

            # ---- stage 4: output projection ----
            for e in range(4):
                for bi in range(2):
                    py = psp.tile([128, 512], f32, tag="ps", name="py")
                    for m in range(8):
                        nc.tensor.matmul(
                            py[:], onorm[:, m, bi * 128:(bi + 1) * 128],
                            wcts[:, m, e * 512:(e + 1) * 512],
                            start=(m == 0), stop=(m == 7))
                    ye = evp.tile([128, 512], f32, tag="ye", name="ye")
                    if (e + bi) % 2 == 0:
                        nc.vector.tensor_copy(ye[:], py[:])
                    else:
                        nc.scalar.copy(ye[:], py[:])
                    eng = nc.sync if bi == 0 else nc.gpsimd
                    eng.dma_start(out=yout[bi, :, e * 512:(e + 1) * 512], in_=ye[:])

    nc.finalize()
    return nc


def kernel(**inputs):
    if "nc" not in _CACHE:
        _CACHE["nc"] = _build_program()
    nc = _CACHE["nc"]
    from concourse.bass_utils import run_bass_kernel_spmd

    in_maps = _prep(inputs)
    res = run_bass_kernel_spmd(nc, in_maps, list(range(NCORES)))
    y = np.zeros((1, S, DIM), np.float32)
    for c in range(NCORES):
        yc = res.results[c]["y"]
        y[0, c * 128:(c + 1) * 128] = yc[0]
        y[0, (15 - c) * 128:(16 - c) * 128] = yc[1]
    return y
